# revision 9
# baseline (speedup 1.0000x reference)
"""EdgeConv-style GNN message passing kernel for 8 TRN2 NeuronCores.

Computation (per edge e with endpoints row[e], col[e]):
    out0 = edge_attr @ w_self
    out  = out0 * (1 + 0.5*(x[row] @ w_h) + 0.5*(x[col] @ w_t)) + edge_attr
    out  = relu(batchnorm(out))          # BN stats over ALL edges (training mode)

Design (v1, rewritten from the fp32 lo/hi-gather baseline):

- Edges are sharded contiguously across the 8 cores; within each core the
  HOST sorts edges into 4 classes by (row < 32768, col < 32768) and pads
  each class segment to a multiple of 2048 with dummy edges (ea = 0 so
  they contribute exactly 0 to the BN sums).  Each segment uses a single
  gather window per side (xb[0:32768] or xb[7232:40000]) so every int16
  index is valid: no zero-row double-fetch, half the gather traffic of
  the lo/hi scheme.  The host un-permutes the output rows at the end.

- Gathers use SWDGE dma_gather with transpose=True on a bf16 copy of x:
  gathered data lands CHANNEL-major ([c, e]) directly, eliminating all
  per-tile PE transposes.  Gathers rotate across 4 SWDGE queues so
  descriptor generation is not ring-credit serialized on gpsimd.

- All matmuls run in bf16 (weights host-cast; 0.5 folded into w_h/w_t).
  edge_attr is supplied channel-major bf16 (host transpose) so it feeds
  the w_self matmul as rhs directly and the residual add as-is.

- Per 512-edge subchunk: s = 0.5*wh@gh + 0.5*wt@gt accumulates in one
  PSUM bank; out0 in another; a = s+1 (ACT copy w/ bias); m = out0*a
  (DVE); out_pre = m + eaT with the per-channel SUM fused in one DVE
  tensor_tensor_reduce; sum-of-squares via ACT Square accum_out.
  out_pre (bf16, channel-major) streams to a DRAM scratch.

- BN stats AllReduce across cores, then pass 2: reload scratch, one ACT
  relu(scale*x+bias) per chunk, store channel-major bf16 output.  Host
  transposes back to [E, C], un-permutes, and upcasts to fp32.
"""

import numpy as np
import ml_dtypes

import concourse.bass as bass
import concourse.mybir as mybir
import concourse.tile as tile
from concourse import bacc

P = 128
C = 128
BN_EPS = 1e-5

N_CORES = 8
N_NODES = 40000
N_EDGES = 640000
E_SHARD = N_EDGES // N_CORES  # 80000

CHUNK = 2048          # edges per gather/DMA chunk (all chunks full-size)
SUB = 512             # edges per compute subchunk (one PSUM bank fp32)

LO_ROWS = 32768       # lo window = xb[0:32768]
HI_BASE = N_NODES - LO_ROWS  # 7232; hi window = xb[7232:40000]

F32 = mybir.dt.float32
BF16 = mybir.dt.bfloat16
I16 = mybir.dt.int16
AF = mybir.ActivationFunctionType
ALU = mybir.AluOpType

BF = ml_dtypes.bfloat16


def build_nc(seg_chunks, n_cores=N_CORES, n_edges_total=N_EDGES):
    """seg_chunks: tuple of 4 ints — number of 2048-edge chunks per class
    segment (uniform across cores)."""
    nchunk = sum(seg_chunks)
    e_pad = nchunk * CHUNK
    nsub = e_pad // SUB
    S = CHUNK // 16  # idx columns per chunk

    nc = bacc.Bacc(None, num_devices=n_cores)
    xb_t = nc.dram_tensor("xb", [N_NODES, C], BF16, kind="ExternalInput")
    eaT_t = nc.dram_tensor("eaT", [C, e_pad], BF16, kind="ExternalInput")
    idx_t = nc.dram_tensor("idxpack", [nchunk, 2, P, S], I16,
                           kind="ExternalInput")
    ws_t = nc.dram_tensor("w_self", [C, C], BF16, kind="ExternalInput")
    wh_t = nc.dram_tensor("w_h2", [C, C], BF16, kind="ExternalInput")
    wt_t = nc.dram_tensor("w_t2", [C, C], BF16, kind="ExternalInput")
    gm_t = nc.dram_tensor("gamma", [C, 1], F32, kind="ExternalInput")
    bt_t = nc.dram_tensor("beta", [C, 1], F32, kind="ExternalInput")
    out_t = nc.dram_tensor("outT", [C, e_pad], BF16, kind="ExternalOutput")

    # window views for the gathers, per class (row_win, col_win)
    def win(lo):
        return slice(0, LO_ROWS) if lo else slice(HI_BASE, N_NODES)

    with tile.TileContext(nc, num_cores=n_cores) as tc:
        with (
            tc.tile_pool(name="constp", bufs=1) as constp,
            tc.tile_pool(name="dramp", bufs=1, space="DRAM") as dramp,
        ):
            w_self_sb = constp.tile([P, C], BF16)
            nc.sync.dma_start(w_self_sb[:], ws_t[:, :])
            wh2 = constp.tile([P, C], BF16)
            nc.sync.dma_start(wh2[:], wh_t[:, :])
            wt2 = constp.tile([P, C], BF16)
            nc.sync.dma_start(wt2[:], wt_t[:, :])
            gamma_sb = constp.tile([P, 1], F32)
            nc.sync.dma_start(gamma_sb[:], gm_t[:, :])
            beta_sb = constp.tile([P, 1], F32)
            nc.sync.dma_start(beta_sb[:], bt_t[:, :])

            sum_cols = constp.tile([P, nsub], F32)
            sq_cols = constp.tile([P, nsub], F32)

            scr = dramp.tile([C, e_pad], BF16)

            # ---- pass 1 ----
            t_idx = 0
            gi = 0
            with (
                tc.tile_pool(name="chunkp", bufs=3) as chunkp,
                tc.tile_pool(name="subp", bufs=3) as subp,
                tc.tile_pool(name="psp", bufs=2, space="PSUM") as psp,
            ):
                for cls in range(4):
                    row_lo, col_lo = cls < 2, cls % 2 == 0
                    xw_r = xb_t[win(row_lo), :]
                    xw_c = xb_t[win(col_lo), :]
                    for _ in range(seg_chunks[cls]):
                        off = gi * CHUNK
                        idx = chunkp.tile([P, 2, S], I16, tag="idx")
                        nc.sync.dma_start(
                            idx[:],
                            idx_t[gi, :, :, :].rearrange("j p s -> p j s"),
                        )
                        if row_lo == col_lo:
                            # same window for both sides: one merged gather
                            # (idx [P, 2, S] flattens to rows-wrap ‖ cols-wrap)
                            g2 = chunkp.tile([P, 1, 2 * CHUNK], BF16, tag="g2")
                            nc.gpsimd.dma_gather(
                                out_ap=g2[:], in_ap=xw_r,
                                idxs_ap=idx[:].rearrange("p j s -> p (j s)"),
                                num_idxs=2 * CHUNK, num_idxs_reg=2 * CHUNK,
                                elem_size=C, transpose=True,
                                single_packet=False,
                            )
                            gh = g2[:, :, 0:CHUNK]
                            gt = g2[:, :, CHUNK:2 * CHUNK]
                        else:
                            ghT = chunkp.tile([P, 1, CHUNK], BF16, tag="gh")
                            gtT = chunkp.tile([P, 1, CHUNK], BF16, tag="gt")
                            nc.gpsimd.dma_gather(
                                out_ap=ghT[:], in_ap=xw_r, idxs_ap=idx[:, 0, :],
                                num_idxs=CHUNK, num_idxs_reg=CHUNK, elem_size=C,
                                transpose=True, single_packet=False,
                            )
                            nc.gpsimd.dma_gather(
                                out_ap=gtT[:], in_ap=xw_c, idxs_ap=idx[:, 1, :],
                                num_idxs=CHUNK, num_idxs_reg=CHUNK, elem_size=C,
                                transpose=True, single_packet=False,
                            )
                            gh, gt = ghT[:], gtT[:]
                        ea_c = chunkp.tile([P, CHUNK], BF16, tag="eac")
                        nc.sync.dma_start(ea_c[:], eaT_t[:, off:off + CHUNK])

                        for s0 in range(0, CHUNK, SUB):
                            sl = slice(s0, s0 + SUB)
                            s_ps = psp.tile([P, SUB], F32, tag="sps", bufs=2)
                            nc.tensor.matmul(
                                s_ps[:], lhsT=wh2[:], rhs=gh[:, 0, sl],
                                start=True, stop=False,
                            )
                            nc.tensor.matmul(
                                s_ps[:], lhsT=wt2[:], rhs=gt[:, 0, sl],
                                start=False, stop=True,
                            )
                            o_ps = psp.tile([P, SUB], F32, tag="ops", bufs=2)
                            nc.tensor.matmul(
                                o_ps[:], lhsT=w_self_sb[:], rhs=ea_c[:, sl],
                                start=True, stop=True,
                            )
                            a1 = subp.tile([P, SUB], F32, tag="a1")
                            nc.scalar.activation(a1[:], s_ps[:], AF.Copy, bias=1.0)
                            m = subp.tile([P, SUB], F32, tag="m")
                            nc.vector.tensor_tensor(m[:], o_ps[:], a1[:], op=ALU.mult)
                            opT = subp.tile([P, SUB], BF16, tag="opT")
                            nc.vector.tensor_tensor(
                                opT[:], m[:], ea_c[:, sl], op=ALU.add
                            )
                            nc.vector.tensor_reduce(
                                sum_cols[:, t_idx:t_idx + 1], opT[:],
                                axis=mybir.AxisListType.X, op=ALU.add,
                            )
                            sqd = subp.tile([P, SUB], BF16, tag="sqd")
                            nc.scalar.activation(
                                sqd[:], opT[:], AF.Square,
                                accum_out=sq_cols[:, t_idx:t_idx + 1],
                            )
                            nc.sync.dma_start(scr[:, off + s0:off + s0 + SUB], opT[:])
                            t_idx += 1
                        gi += 1
            assert t_idx == nsub and gi == nchunk

            # ---- BN stats all-reduce + scale/shift ----
            stats2 = constp.tile([P, 2], F32)
            nc.vector.tensor_reduce(
                stats2[:, 0:1], sum_cols[:], axis=mybir.AxisListType.X, op=ALU.add
            )
            nc.vector.tensor_reduce(
                stats2[:, 1:2], sq_cols[:], axis=mybir.AxisListType.X, op=ALU.add
            )
            cc_in = dramp.tile([P, 2], F32)
            nc.sync.dma_start(cc_in[:], stats2[:])
            cc_addr = "Shared" if n_cores > 4 else "Local"
            cc_out = dramp.tile([P, 2], F32, addr_space=cc_addr)
            nc.gpsimd.collective_compute(
                "AllReduce",
                ALU.add,
                replica_groups=[list(range(n_cores))],
                ins=[cc_in[:].opt()],
                outs=[cc_out[:].opt()],
            )
            statsg = constp.tile([P, 2], F32)
            nc.sync.dma_start(statsg[:], cc_out[:])

            inv_e = 1.0 / float(n_edges_total)
            mean = constp.tile([P, 1], F32)
            nc.scalar.mul(mean[:], statsg[:, 0:1], inv_e)
            ex2 = constp.tile([P, 1], F32)
            nc.scalar.mul(ex2[:], statsg[:, 1:2], inv_e)
            msq = constp.tile([P, 1], F32)
            nc.vector.tensor_tensor(msq[:], mean[:], mean[:], op=ALU.mult)
            var = constp.tile([P, 1], F32)
            nc.vector.tensor_tensor(var[:], ex2[:], msq[:], op=ALU.subtract)
            eps_sb = constp.tile([P, 1], F32)
            nc.gpsimd.memset(eps_sb[:], BN_EPS)
            std = constp.tile([P, 1], F32)
            nc.scalar.activation(std[:], var[:], AF.Sqrt, bias=eps_sb[:])
            rstd = constp.tile([P, 1], F32)
            nc.vector.reciprocal(rstd[:], std[:])
            scale = constp.tile([P, 1], F32)
            nc.vector.tensor_tensor(scale[:], gamma_sb[:], rstd[:], op=ALU.mult)
            mscale = constp.tile([P, 1], F32)
            nc.vector.tensor_tensor(mscale[:], mean[:], scale[:], op=ALU.mult)
            shift = constp.tile([P, 1], F32)
            nc.vector.tensor_tensor(shift[:], beta_sb[:], mscale[:], op=ALU.subtract)

            # ---- pass 2: relu(scale*x + shift), channel-major ----
            P2W = 4 * CHUNK
            with tc.tile_pool(name="p2p", bufs=3) as p2p:
                for off in range(0, e_pad, P2W):
                    w = min(P2W, e_pad - off)
                    opn = p2p.tile([P, w], BF16, tag="opn")
                    nc.sync.dma_start(opn[:], scr[:, off:off + w])
                    nrm = p2p.tile([P, w], BF16, tag="nrm")
                    nc.scalar.activation(
                        nrm[:], opn[:], AF.Relu, bias=shift[:], scale=scale[:]
                    )
                    nc.sync.dma_start(out_t[:, off:off + w], nrm[:])

    if not nc.is_finalized():
        nc.finalize()
    return nc


def _wrap16(a, S):
    """[n] int array -> dma_gather idx layout [128, S] int16 (zero-pad to 16*S)."""
    out = np.zeros((16, S), dtype=np.int16)
    n = a.shape[0]
    full = np.zeros(16 * S, dtype=np.int16)
    full[:n] = a.astype(np.int16)
    out = full.reshape(S, 16).T
    return np.tile(out, (8, 1))


def prepare(x, edge_index, edge_attr, w_self, w_h, w_t, gamma, beta_bn):
    """Host-side packing.  Returns (seg_chunks, in_maps, restore) where
    restore[core] = (positions array mapping padded row -> original edge)."""
    x = np.asarray(x, dtype=np.float32)
    xb = x.astype(BF)

    ei = np.asarray(edge_index)
    row_all = ei[0].astype(np.int64)
    col_all = ei[1].astype(np.int64)
    ea_all = np.asarray(edge_attr, dtype=np.float32)

    ws = np.ascontiguousarray(np.asarray(w_self, dtype=np.float32)).astype(BF)
    wh = (0.5 * np.asarray(w_h, dtype=np.float32)).astype(BF)
    wt = (0.5 * np.asarray(w_t, dtype=np.float32)).astype(BF)
    gm = np.ascontiguousarray(np.asarray(gamma, np.float32).reshape(C, 1))
    bt = np.ascontiguousarray(np.asarray(beta_bn, np.float32).reshape(C, 1))

    # per-core class split
    cores = []
    counts = np.zeros((N_CORES, 4), dtype=np.int64)
    for c in range(N_CORES):
        sl = slice(c * E_SHARD, (c + 1) * E_SHARD)
        row, col = row_all[sl], col_all[sl]
        cls = 2 * (row >= LO_ROWS).astype(np.int64) + (col >= LO_ROWS)
        order = np.argsort(cls, kind="stable")
        cores.append((row, col, cls, order, ea_all[sl]))
        counts[c] = np.bincount(cls, minlength=4)

    seg_chunks = tuple(
        int(-(-int(counts[:, k].max()) // CHUNK)) for k in range(4)
    )
    nchunk = sum(seg_chunks)
    e_pad = nchunk * CHUNK
    S = CHUNK // 16
    seg_off = np.concatenate([[0], np.cumsum(np.array(seg_chunks) * CHUNK)])

    in_maps = []
    restores = []
    for c in range(N_CORES):
        row, col, cls, order, ea = cores[c]
        ridx = np.zeros(e_pad, dtype=np.int64)
        cidx = np.zeros(e_pad, dtype=np.int64)
        eaP = np.zeros((e_pad, C), dtype=np.float32)
        rest = []  # (orig_ids, padded_start, n)
        for k in range(4):
            ids = order[cls[order] == k]
            n = len(ids)
            o0 = int(seg_off[k])
            r = row[ids] - (0 if k < 2 else HI_BASE)
            cc = col[ids] - (0 if k % 2 == 0 else HI_BASE)
            ridx[o0:o0 + n] = r
            cidx[o0:o0 + n] = cc
            eaP[o0:o0 + n] = ea[ids]
            rest.append((ids, o0, n))
        restores.append(rest)

        idxpack = np.zeros((nchunk, 2, P, S), dtype=np.int16)
        for giq in range(nchunk):
            a, b = giq * CHUNK, (giq + 1) * CHUNK
            idxpack[giq, 0] = _wrap16(ridx[a:b], S)
            idxpack[giq, 1] = _wrap16(cidx[a:b], S)

        eaT = np.ascontiguousarray(eaP.T.astype(BF))
        in_maps.append({
            "xb": xb,
            "eaT": eaT,
            "idxpack": idxpack,
            "w_self": ws,
            "w_h2": wh,
            "w_t2": wt,
            "gamma": gm,
            "beta": bt,
        })
    return seg_chunks, in_maps, restores


_NC_CACHE = {}


def _get_nc(seg_chunks):
    if seg_chunks not in _NC_CACHE:
        _NC_CACHE[seg_chunks] = build_nc(seg_chunks)
    return _NC_CACHE[seg_chunks]


def run(inputs, trace=False, **kwargs):
    from concourse.bass_utils import run_bass_kernel_spmd

    seg_chunks, in_maps, restores = prepare(
        inputs["x"], inputs["edge_index"], inputs["edge_attr"],
        inputs["w_self"], inputs["w_h"], inputs["w_t"],
        inputs["gamma"], inputs["beta_bn"],
    )
    nc = _get_nc(seg_chunks)
    res = run_bass_kernel_spmd(
        nc, in_maps, core_ids=list(range(N_CORES)), trace=trace, **kwargs
    )
    out = np.empty((N_EDGES, C), dtype=np.float32)
    for c in range(N_CORES):
        outT = np.asarray(res.results[c]["outT"])  # [C, e_pad] bf16
        outP = outT.T.astype(np.float32)           # [e_pad, C]
        base = c * E_SHARD
        for ids, o0, n in restores[c]:
            out[base + ids] = outP[o0:o0 + n]
    return out, res


def kernel(**inputs):
    out, _ = run(inputs, trace=False)
    return out


# revision 10
# speedup vs baseline: 1.0262x; 1.0262x over previous
"""EdgeConv-style GNN message passing kernel for 8 TRN2 NeuronCores.

Computation (per edge e with endpoints row[e], col[e]):
    out0 = edge_attr @ w_self
    out  = out0 * (1 + 0.5*(x[row] @ w_h) + 0.5*(x[col] @ w_t)) + edge_attr
    out  = relu(batchnorm(out))          # BN stats over ALL edges (training mode)

Design (v1, rewritten from the fp32 lo/hi-gather baseline):

- Edges are sharded contiguously across the 8 cores; within each core the
  HOST sorts edges into 4 classes by (row < 32768, col < 32768) and pads
  each class segment to a multiple of 2048 with dummy edges (ea = 0 so
  they contribute exactly 0 to the BN sums).  Each segment uses a single
  gather window per side (xb[0:32768] or xb[7232:40000]) so every int16
  index is valid: no zero-row double-fetch, half the gather traffic of
  the lo/hi scheme.  The host un-permutes the output rows at the end.

- Gathers use SWDGE dma_gather with transpose=True on a bf16 copy of x:
  gathered data lands CHANNEL-major ([c, e]) directly, eliminating all
  per-tile PE transposes.  Gathers rotate across 4 SWDGE queues so
  descriptor generation is not ring-credit serialized on gpsimd.

- All matmuls run in bf16 (weights host-cast; 0.5 folded into w_h/w_t).
  edge_attr is supplied channel-major bf16 (host transpose) so it feeds
  the w_self matmul as rhs directly and the residual add as-is.

- Per 512-edge subchunk: s = 0.5*wh@gh + 0.5*wt@gt accumulates in one
  PSUM bank; out0 in another; a = s+1 (ACT copy w/ bias); m = out0*a
  (DVE); out_pre = m + eaT with the per-channel SUM fused in one DVE
  tensor_tensor_reduce; sum-of-squares via ACT Square accum_out.
  out_pre (bf16, channel-major) streams to a DRAM scratch.

- BN stats AllReduce across cores, then pass 2: reload scratch, one ACT
  relu(scale*x+bias) per chunk, store channel-major bf16 output.  Host
  transposes back to [E, C], un-permutes, and upcasts to fp32.
"""

import numpy as np
import ml_dtypes

import concourse.bass as bass
import concourse.mybir as mybir
import concourse.tile as tile
from concourse import bacc

P = 128
C = 128
BN_EPS = 1e-5

N_CORES = 8
N_NODES = 40000
N_EDGES = 640000
E_SHARD = N_EDGES // N_CORES  # 80000

CHUNK = 2048          # edges per gather/DMA chunk (all chunks full-size)
SUB = 512             # edges per compute subchunk (one PSUM bank fp32)

LO_ROWS = 32768       # lo window = xb[0:32768]
HI_BASE = N_NODES - LO_ROWS  # 7232; hi window = xb[7232:40000]

F32 = mybir.dt.float32
BF16 = mybir.dt.bfloat16
I16 = mybir.dt.int16
AF = mybir.ActivationFunctionType
ALU = mybir.AluOpType

BF = ml_dtypes.bfloat16


def build_nc(seg_chunks, n_cores=N_CORES, n_edges_total=N_EDGES):
    """seg_chunks: tuple of 4 ints — number of 2048-edge chunks per class
    segment (uniform across cores)."""
    nchunk = sum(seg_chunks)
    e_pad = nchunk * CHUNK
    nsub = e_pad // SUB
    S = CHUNK // 16  # idx columns per chunk

    nc = bacc.Bacc(None, num_devices=n_cores)
    xb_t = nc.dram_tensor("xb", [N_NODES, C], BF16, kind="ExternalInput")
    eaT_t = nc.dram_tensor("eaT", [C, e_pad], BF16, kind="ExternalInput")
    idx_t = nc.dram_tensor("idxpack", [nchunk, 2, P, S], I16,
                           kind="ExternalInput")
    ws_t = nc.dram_tensor("w_self", [C, C], BF16, kind="ExternalInput")
    wh_t = nc.dram_tensor("w_h2", [C, C], BF16, kind="ExternalInput")
    wt_t = nc.dram_tensor("w_t2", [C, C], BF16, kind="ExternalInput")
    gm_t = nc.dram_tensor("gamma", [C, 1], F32, kind="ExternalInput")
    bt_t = nc.dram_tensor("beta", [C, 1], F32, kind="ExternalInput")
    out_t = nc.dram_tensor("outT", [C, e_pad], BF16, kind="ExternalOutput")

    # window views for the gathers, per class (row_win, col_win)
    def win(lo):
        return slice(0, LO_ROWS) if lo else slice(HI_BASE, N_NODES)

    with tile.TileContext(nc, num_cores=n_cores) as tc:
        with (
            tc.tile_pool(name="constp", bufs=1) as constp,
            tc.tile_pool(name="dramp", bufs=1, space="DRAM") as dramp,
        ):
            w_self_sb = constp.tile([P, C], BF16)
            nc.sync.dma_start(w_self_sb[:], ws_t[:, :])
            wh2 = constp.tile([P, C], BF16)
            nc.sync.dma_start(wh2[:], wh_t[:, :])
            wt2 = constp.tile([P, C], BF16)
            nc.sync.dma_start(wt2[:], wt_t[:, :])
            gamma_sb = constp.tile([P, 1], F32)
            nc.sync.dma_start(gamma_sb[:], gm_t[:, :])
            beta_sb = constp.tile([P, 1], F32)
            nc.sync.dma_start(beta_sb[:], bt_t[:, :])

            sum_cols = constp.tile([P, nsub], F32)
            sq_cols = constp.tile([P, nsub], F32)

            scr = dramp.tile([C, e_pad], BF16)

            # ---- pass 1 ----
            t_idx = 0
            gi = 0
            with (
                tc.tile_pool(name="chunkp", bufs=3) as chunkp,
                tc.tile_pool(name="subp", bufs=3) as subp,
                tc.tile_pool(name="psp", bufs=2, space="PSUM") as psp,
            ):
                for cls in range(4):
                    row_lo, col_lo = cls < 2, cls % 2 == 0
                    xw_r = xb_t[win(row_lo), :]
                    xw_c = xb_t[win(col_lo), :]
                    for _ in range(seg_chunks[cls]):
                        off = gi * CHUNK
                        idx = chunkp.tile([P, 2, S], I16, tag="idx")
                        nc.sync.dma_start(
                            idx[:],
                            idx_t[gi, :, :, :].rearrange("j p s -> p j s"),
                        )
                        gh = chunkp.tile([P, 1, CHUNK], BF16, tag="gh")
                        gt = chunkp.tile([P, 1, CHUNK], BF16, tag="gt")
                        nc.gpsimd.dma_gather(
                            out_ap=gh[:], in_ap=xw_r, idxs_ap=idx[:, 0, :],
                            num_idxs=CHUNK, num_idxs_reg=CHUNK, elem_size=C,
                            transpose=True, single_packet=False,
                        )
                        nc.gpsimd.dma_gather(
                            out_ap=gt[:], in_ap=xw_c, idxs_ap=idx[:, 1, :],
                            num_idxs=CHUNK, num_idxs_reg=CHUNK, elem_size=C,
                            transpose=True, single_packet=False,
                        )
                        ea_c = chunkp.tile([P, CHUNK], BF16, tag="eac")
                        nc.sync.dma_start(ea_c[:], eaT_t[:, off:off + CHUNK])

                        for s0 in range(0, CHUNK, SUB):
                            sl = slice(s0, s0 + SUB)
                            s_ps = psp.tile([P, SUB], F32, tag="sps", bufs=2)
                            nc.tensor.matmul(
                                s_ps[:], lhsT=wh2[:], rhs=gh[:, 0, sl],
                                start=True, stop=False,
                            )
                            nc.tensor.matmul(
                                s_ps[:], lhsT=wt2[:], rhs=gt[:, 0, sl],
                                start=False, stop=True,
                            )
                            o_ps = psp.tile([P, SUB], F32, tag="ops", bufs=2)
                            nc.tensor.matmul(
                                o_ps[:], lhsT=w_self_sb[:], rhs=ea_c[:, sl],
                                start=True, stop=True,
                            )
                            a1 = subp.tile([P, SUB], F32, tag="a1")
                            nc.scalar.activation(a1[:], s_ps[:], AF.Copy, bias=1.0)
                            m = subp.tile([P, SUB], F32, tag="m")
                            nc.vector.tensor_tensor(m[:], o_ps[:], a1[:], op=ALU.mult)
                            opT = subp.tile([P, SUB], BF16, tag="opT")
                            nc.vector.tensor_tensor(
                                opT[:], m[:], ea_c[:, sl], op=ALU.add
                            )
                            nc.vector.tensor_reduce(
                                sum_cols[:, t_idx:t_idx + 1], opT[:],
                                axis=mybir.AxisListType.X, op=ALU.add,
                            )
                            sqd = subp.tile([P, SUB], BF16, tag="sqd")
                            nc.scalar.activation(
                                sqd[:], opT[:], AF.Square,
                                accum_out=sq_cols[:, t_idx:t_idx + 1],
                            )
                            nc.sync.dma_start(scr[:, off + s0:off + s0 + SUB], opT[:])
                            t_idx += 1
                        gi += 1
            assert t_idx == nsub and gi == nchunk

            # ---- BN stats all-reduce + scale/shift ----
            stats2 = constp.tile([P, 2], F32)
            nc.vector.tensor_reduce(
                stats2[:, 0:1], sum_cols[:], axis=mybir.AxisListType.X, op=ALU.add
            )
            nc.vector.tensor_reduce(
                stats2[:, 1:2], sq_cols[:], axis=mybir.AxisListType.X, op=ALU.add
            )
            cc_in = dramp.tile([P, 2], F32)
            nc.sync.dma_start(cc_in[:], stats2[:])
            cc_addr = "Shared" if n_cores > 4 else "Local"
            cc_out = dramp.tile([P, 2], F32, addr_space=cc_addr)
            nc.gpsimd.collective_compute(
                "AllReduce",
                ALU.add,
                replica_groups=[list(range(n_cores))],
                ins=[cc_in[:].opt()],
                outs=[cc_out[:].opt()],
            )
            statsg = constp.tile([P, 2], F32)
            nc.sync.dma_start(statsg[:], cc_out[:])

            inv_e = 1.0 / float(n_edges_total)
            mean = constp.tile([P, 1], F32)
            nc.scalar.mul(mean[:], statsg[:, 0:1], inv_e)
            ex2 = constp.tile([P, 1], F32)
            nc.scalar.mul(ex2[:], statsg[:, 1:2], inv_e)
            msq = constp.tile([P, 1], F32)
            nc.vector.tensor_tensor(msq[:], mean[:], mean[:], op=ALU.mult)
            var = constp.tile([P, 1], F32)
            nc.vector.tensor_tensor(var[:], ex2[:], msq[:], op=ALU.subtract)
            eps_sb = constp.tile([P, 1], F32)
            nc.gpsimd.memset(eps_sb[:], BN_EPS)
            std = constp.tile([P, 1], F32)
            nc.scalar.activation(std[:], var[:], AF.Sqrt, bias=eps_sb[:])
            rstd = constp.tile([P, 1], F32)
            nc.vector.reciprocal(rstd[:], std[:])
            scale = constp.tile([P, 1], F32)
            nc.vector.tensor_tensor(scale[:], gamma_sb[:], rstd[:], op=ALU.mult)
            mscale = constp.tile([P, 1], F32)
            nc.vector.tensor_tensor(mscale[:], mean[:], scale[:], op=ALU.mult)
            shift = constp.tile([P, 1], F32)
            nc.vector.tensor_tensor(shift[:], beta_sb[:], mscale[:], op=ALU.subtract)

            # ---- pass 2: relu(scale*x + shift), channel-major ----
            P2W = 4 * CHUNK
            with tc.tile_pool(name="p2p", bufs=3) as p2p:
                for off in range(0, e_pad, P2W):
                    w = min(P2W, e_pad - off)
                    opn = p2p.tile([P, w], BF16, tag="opn")
                    nc.sync.dma_start(opn[:], scr[:, off:off + w])
                    nrm = p2p.tile([P, w], BF16, tag="nrm")
                    nc.scalar.activation(
                        nrm[:], opn[:], AF.Relu, bias=shift[:], scale=scale[:]
                    )
                    nc.sync.dma_start(out_t[:, off:off + w], nrm[:])

    if not nc.is_finalized():
        nc.finalize()
    return nc


def _wrap16(a, S):
    """[n] int array -> dma_gather idx layout [128, S] int16 (zero-pad to 16*S)."""
    out = np.zeros((16, S), dtype=np.int16)
    n = a.shape[0]
    full = np.zeros(16 * S, dtype=np.int16)
    full[:n] = a.astype(np.int16)
    out = full.reshape(S, 16).T
    return np.tile(out, (8, 1))


def prepare(x, edge_index, edge_attr, w_self, w_h, w_t, gamma, beta_bn):
    """Host-side packing.  Returns (seg_chunks, in_maps, restore) where
    restore[core] = (positions array mapping padded row -> original edge)."""
    x = np.asarray(x, dtype=np.float32)
    xb = x.astype(BF)

    ei = np.asarray(edge_index)
    row_all = ei[0].astype(np.int64)
    col_all = ei[1].astype(np.int64)
    ea_all = np.asarray(edge_attr, dtype=np.float32)

    ws = np.ascontiguousarray(np.asarray(w_self, dtype=np.float32)).astype(BF)
    wh = (0.5 * np.asarray(w_h, dtype=np.float32)).astype(BF)
    wt = (0.5 * np.asarray(w_t, dtype=np.float32)).astype(BF)
    gm = np.ascontiguousarray(np.asarray(gamma, np.float32).reshape(C, 1))
    bt = np.ascontiguousarray(np.asarray(beta_bn, np.float32).reshape(C, 1))

    # per-core class split
    cores = []
    counts = np.zeros((N_CORES, 4), dtype=np.int64)
    for c in range(N_CORES):
        sl = slice(c * E_SHARD, (c + 1) * E_SHARD)
        row, col = row_all[sl], col_all[sl]
        cls = 2 * (row >= LO_ROWS).astype(np.int64) + (col >= LO_ROWS)
        order = np.argsort(cls, kind="stable")
        cores.append((row, col, cls, order, ea_all[sl]))
        counts[c] = np.bincount(cls, minlength=4)

    seg_chunks = tuple(
        int(-(-int(counts[:, k].max()) // CHUNK)) for k in range(4)
    )
    nchunk = sum(seg_chunks)
    e_pad = nchunk * CHUNK
    S = CHUNK // 16
    seg_off = np.concatenate([[0], np.cumsum(np.array(seg_chunks) * CHUNK)])

    in_maps = []
    restores = []
    for c in range(N_CORES):
        row, col, cls, order, ea = cores[c]
        ridx = np.zeros(e_pad, dtype=np.int64)
        cidx = np.zeros(e_pad, dtype=np.int64)
        eaP = np.zeros((e_pad, C), dtype=np.float32)
        rest = []  # (orig_ids, padded_start, n)
        for k in range(4):
            ids = order[cls[order] == k]
            n = len(ids)
            o0 = int(seg_off[k])
            r = row[ids] - (0 if k < 2 else HI_BASE)
            cc = col[ids] - (0 if k % 2 == 0 else HI_BASE)
            ridx[o0:o0 + n] = r
            cidx[o0:o0 + n] = cc
            eaP[o0:o0 + n] = ea[ids]
            rest.append((ids, o0, n))
        restores.append(rest)

        idxpack = np.zeros((nchunk, 2, P, S), dtype=np.int16)
        for giq in range(nchunk):
            a, b = giq * CHUNK, (giq + 1) * CHUNK
            idxpack[giq, 0] = _wrap16(ridx[a:b], S)
            idxpack[giq, 1] = _wrap16(cidx[a:b], S)

        eaT = np.ascontiguousarray(eaP.T.astype(BF))
        in_maps.append({
            "xb": xb,
            "eaT": eaT,
            "idxpack": idxpack,
            "w_self": ws,
            "w_h2": wh,
            "w_t2": wt,
            "gamma": gm,
            "beta": bt,
        })
    return seg_chunks, in_maps, restores


_NC_CACHE = {}


def _get_nc(seg_chunks):
    if seg_chunks not in _NC_CACHE:
        _NC_CACHE[seg_chunks] = build_nc(seg_chunks)
    return _NC_CACHE[seg_chunks]


def run(inputs, trace=False, **kwargs):
    from concourse.bass_utils import run_bass_kernel_spmd

    seg_chunks, in_maps, restores = prepare(
        inputs["x"], inputs["edge_index"], inputs["edge_attr"],
        inputs["w_self"], inputs["w_h"], inputs["w_t"],
        inputs["gamma"], inputs["beta_bn"],
    )
    nc = _get_nc(seg_chunks)
    res = run_bass_kernel_spmd(
        nc, in_maps, core_ids=list(range(N_CORES)), trace=trace, **kwargs
    )
    out = np.empty((N_EDGES, C), dtype=np.float32)
    for c in range(N_CORES):
        outT = np.asarray(res.results[c]["outT"])  # [C, e_pad] bf16
        outP = outT.T.astype(np.float32)           # [e_pad, C]
        base = c * E_SHARD
        for ids, o0, n in restores[c]:
            out[base + ids] = outP[o0:o0 + n]
    return out, res


def kernel(**inputs):
    out, _ = run(inputs, trace=False)
    return out


# revision 19
# speedup vs baseline: 1.0608x; 1.0337x over previous
"""EdgeConv-style GNN message passing kernel for 8 TRN2 NeuronCores.

Computation (per edge e with endpoints row[e], col[e]):
    out0 = edge_attr @ w_self
    out  = out0 * (1 + 0.5*(x[row] @ w_h) + 0.5*(x[col] @ w_t)) + edge_attr
    out  = relu(batchnorm(out))          # BN stats over ALL edges (training mode)

Design (v1, rewritten from the fp32 lo/hi-gather baseline):

- Edges are sharded contiguously across the 8 cores; within each core the
  HOST sorts edges into 4 classes by (row < 32768, col < 32768) and pads
  each class segment to a multiple of 2048 with dummy edges (ea = 0 so
  they contribute exactly 0 to the BN sums).  Each segment uses a single
  gather window per side (xb[0:32768] or xb[7232:40000]) so every int16
  index is valid: no zero-row double-fetch, half the gather traffic of
  the lo/hi scheme.  The host un-permutes the output rows at the end.

- Gathers use SWDGE dma_gather with transpose=True on a bf16 copy of x:
  gathered data lands CHANNEL-major ([c, e]) directly, eliminating all
  per-tile PE transposes.  Gathers rotate across 4 SWDGE queues so
  descriptor generation is not ring-credit serialized on gpsimd.

- All matmuls run in bf16 (weights host-cast; 0.5 folded into w_h/w_t).
  edge_attr is supplied channel-major bf16 (host transpose) so it feeds
  the w_self matmul as rhs directly and the residual add as-is.

- Per 512-edge subchunk: s = 0.5*wh@gh + 0.5*wt@gt accumulates in one
  PSUM bank; out0 in another; a = s+1 (ACT copy w/ bias); m = out0*a
  (DVE); out_pre = m + eaT with the per-channel SUM fused in one DVE
  tensor_tensor_reduce; sum-of-squares via ACT Square accum_out.
  out_pre (bf16, channel-major) streams to a DRAM scratch.

- BN stats AllReduce across cores, then pass 2: reload scratch, one ACT
  relu(scale*x+bias) per chunk, store channel-major bf16 output.  Host
  transposes back to [E, C], un-permutes, and upcasts to fp32.
"""

import numpy as np
import ml_dtypes

import concourse.bass as bass
import concourse.mybir as mybir
import concourse.tile as tile
from concourse import bacc

P = 128
C = 128
BN_EPS = 1e-5

N_CORES = 8
N_NODES = 40000
N_EDGES = 640000
E_SHARD = N_EDGES // N_CORES  # 80000

CHUNK = 2048          # edges per gather/DMA chunk (all chunks full-size)
SUB = 512             # edges per compute subchunk (one PSUM bank fp32)

LO_ROWS = 32768       # lo window = xb[0:32768]
HI_BASE = N_NODES - LO_ROWS  # 7232; hi window = xb[7232:40000]

F32 = mybir.dt.float32
BF16 = mybir.dt.bfloat16
I16 = mybir.dt.int16
AF = mybir.ActivationFunctionType
ALU = mybir.AluOpType

BF = ml_dtypes.bfloat16


def build_nc(seg_sizes, n_cores=N_CORES, n_edges_total=N_EDGES):
    """seg_sizes: tuple of 4 ints — padded edges per class segment
    (multiples of SUB, uniform across cores)."""
    assert all(s % SUB == 0 for s in seg_sizes)
    chunks = []  # (cls, ch)
    for cls in range(4):
        rem = seg_sizes[cls]
        while rem > 0:
            ch = min(CHUNK, rem)
            chunks.append((cls, ch))
            rem -= ch
    nchunk = len(chunks)
    e_pad = sum(seg_sizes)
    nsub = e_pad // SUB
    SMAX = CHUNK // 16  # max idx columns per chunk

    nc = bacc.Bacc(None, num_devices=n_cores)
    xb_t = nc.dram_tensor("xb", [N_NODES, C], BF16, kind="ExternalInput")
    eaT_t = nc.dram_tensor("eaT", [C, e_pad], BF16, kind="ExternalInput")
    idx_t = nc.dram_tensor("idxpack", [nchunk, 2, P, SMAX], I16,
                           kind="ExternalInput")
    ws_t = nc.dram_tensor("w_self", [C, C], BF16, kind="ExternalInput")
    wh_t = nc.dram_tensor("w_h2", [C, C], BF16, kind="ExternalInput")
    wt_t = nc.dram_tensor("w_t2", [C, C], BF16, kind="ExternalInput")
    gm_t = nc.dram_tensor("gamma", [C, 1], F32, kind="ExternalInput")
    bt_t = nc.dram_tensor("beta", [C, 1], F32, kind="ExternalInput")
    out_t = nc.dram_tensor("outT", [C, e_pad], BF16, kind="ExternalOutput")

    # window views for the gathers, per class (row_win, col_win)
    def win(lo):
        return slice(0, LO_ROWS) if lo else slice(HI_BASE, N_NODES)

    with tile.TileContext(nc, num_cores=n_cores) as tc:
        with (
            tc.tile_pool(name="constp", bufs=1) as constp,
            tc.tile_pool(name="dramp", bufs=1, space="DRAM") as dramp,
        ):
            w_self_sb = constp.tile([P, C], BF16)
            nc.sync.dma_start(w_self_sb[:], ws_t[:, :])
            wh2 = constp.tile([P, C], BF16)
            nc.sync.dma_start(wh2[:], wh_t[:, :])
            wt2 = constp.tile([P, C], BF16)
            nc.sync.dma_start(wt2[:], wt_t[:, :])
            gamma_sb = constp.tile([P, 1], F32)
            nc.sync.dma_start(gamma_sb[:], gm_t[:, :])
            beta_sb = constp.tile([P, 1], F32)
            nc.sync.dma_start(beta_sb[:], bt_t[:, :])

            sum_cols = constp.tile([P, nsub], F32)
            sq_cols = constp.tile([P, nsub], F32)

            scr = dramp.tile([C, e_pad], BF16)

            # ---- pass 1 ----
            t_idx = 0
            off = 0
            with (
                tc.tile_pool(name="chunkp", bufs=4) as chunkp,
                tc.tile_pool(name="subp", bufs=3) as subp,
                tc.tile_pool(name="psp", bufs=2, space="PSUM") as psp,
            ):
                for gi, (cls, ch) in enumerate(chunks):
                    row_lo, col_lo = cls < 2, cls % 2 == 0
                    xw_r = xb_t[win(row_lo), :]
                    xw_c = xb_t[win(col_lo), :]
                    S = ch // 16
                    if True:
                        idx = chunkp.tile([P, 2, S], I16, tag="idx")
                        nc.sync.dma_start(
                            idx[:],
                            idx_t[gi, :, :, 0:S].rearrange("j p s -> p j s"),
                        )
                        gh = chunkp.tile([P, 1, ch], BF16, tag="gh")
                        gt = chunkp.tile([P, 1, ch], BF16, tag="gt")
                        nc.gpsimd.dma_gather(
                            out_ap=gh[:], in_ap=xw_r, idxs_ap=idx[:, 0, :],
                            num_idxs=ch, num_idxs_reg=ch, elem_size=C,
                            transpose=True, single_packet=False,
                        )
                        nc.gpsimd.dma_gather(
                            out_ap=gt[:], in_ap=xw_c, idxs_ap=idx[:, 1, :],
                            num_idxs=ch, num_idxs_reg=ch, elem_size=C,
                            transpose=True, single_packet=False,
                        )
                        ea_c = chunkp.tile([P, ch], BF16, tag="eac")
                        nc.sync.dma_start(ea_c[:], eaT_t[:, off:off + ch])

                        for s0 in range(0, ch, SUB):
                            sl = slice(s0, s0 + SUB)
                            s_ps = psp.tile([P, SUB], F32, tag="sps", bufs=2)
                            nc.tensor.matmul(
                                s_ps[:], lhsT=wh2[:], rhs=gh[:, 0, sl],
                                start=True, stop=False,
                            )
                            nc.tensor.matmul(
                                s_ps[:], lhsT=wt2[:], rhs=gt[:, 0, sl],
                                start=False, stop=True,
                            )
                            o_ps = psp.tile([P, SUB], F32, tag="ops", bufs=2)
                            nc.tensor.matmul(
                                o_ps[:], lhsT=w_self_sb[:], rhs=ea_c[:, sl],
                                start=True, stop=True,
                            )
                            a1 = subp.tile([P, SUB], F32, tag="a1")
                            nc.scalar.activation(a1[:], s_ps[:], AF.Copy, bias=1.0)
                            m = subp.tile([P, SUB], F32, tag="m")
                            nc.vector.tensor_tensor(m[:], o_ps[:], a1[:], op=ALU.mult)
                            opT = subp.tile([P, SUB], BF16, tag="opT")
                            nc.vector.tensor_tensor(
                                opT[:], m[:], ea_c[:, sl], op=ALU.add
                            )
                            nc.vector.tensor_reduce(
                                sum_cols[:, t_idx:t_idx + 1], opT[:],
                                axis=mybir.AxisListType.X, op=ALU.add,
                            )
                            sqd = subp.tile([P, SUB], BF16, tag="sqd")
                            nc.scalar.activation(
                                sqd[:], opT[:], AF.Square,
                                accum_out=sq_cols[:, t_idx:t_idx + 1],
                            )
                            nc.sync.dma_start(scr[:, off + s0:off + s0 + SUB], opT[:])
                            t_idx += 1
                    off += ch
            assert t_idx == nsub and off == e_pad

            # ---- BN stats all-reduce + scale/shift ----
            stats2 = constp.tile([P, 2], F32)
            nc.vector.tensor_reduce(
                stats2[:, 0:1], sum_cols[:], axis=mybir.AxisListType.X, op=ALU.add
            )
            nc.vector.tensor_reduce(
                stats2[:, 1:2], sq_cols[:], axis=mybir.AxisListType.X, op=ALU.add
            )
            cc_in = dramp.tile([P, 2], F32)
            nc.sync.dma_start(cc_in[:], stats2[:])
            cc_addr = "Shared" if n_cores > 4 else "Local"
            cc_out = dramp.tile([P, 2], F32, addr_space=cc_addr)
            nc.gpsimd.collective_compute(
                "AllReduce",
                ALU.add,
                replica_groups=[list(range(n_cores))],
                ins=[cc_in[:].opt()],
                outs=[cc_out[:].opt()],
            )
            statsg = constp.tile([P, 2], F32)
            nc.sync.dma_start(statsg[:], cc_out[:])

            inv_e = 1.0 / float(n_edges_total)
            mean = constp.tile([P, 1], F32)
            nc.scalar.mul(mean[:], statsg[:, 0:1], inv_e)
            ex2 = constp.tile([P, 1], F32)
            nc.scalar.mul(ex2[:], statsg[:, 1:2], inv_e)
            msq = constp.tile([P, 1], F32)
            nc.vector.tensor_tensor(msq[:], mean[:], mean[:], op=ALU.mult)
            var = constp.tile([P, 1], F32)
            nc.vector.tensor_tensor(var[:], ex2[:], msq[:], op=ALU.subtract)
            eps_sb = constp.tile([P, 1], F32)
            nc.gpsimd.memset(eps_sb[:], BN_EPS)
            std = constp.tile([P, 1], F32)
            nc.scalar.activation(std[:], var[:], AF.Sqrt, bias=eps_sb[:])
            rstd = constp.tile([P, 1], F32)
            nc.vector.reciprocal(rstd[:], std[:])
            scale = constp.tile([P, 1], F32)
            nc.vector.tensor_tensor(scale[:], gamma_sb[:], rstd[:], op=ALU.mult)
            mscale = constp.tile([P, 1], F32)
            nc.vector.tensor_tensor(mscale[:], mean[:], scale[:], op=ALU.mult)
            shift = constp.tile([P, 1], F32)
            nc.vector.tensor_tensor(shift[:], beta_sb[:], mscale[:], op=ALU.subtract)

            # ---- pass 2: relu(scale*x + shift), channel-major ----
            # Split tiles between ACT (1 fused op) and DVE (affine + max),
            # the two engines stream in parallel.
            P2W = 4 * CHUNK
            with tc.tile_pool(name="p2p", bufs=4) as p2p:
                for ti, off2 in enumerate(range(0, e_pad, P2W)):
                    w = min(P2W, e_pad - off2)
                    opn = p2p.tile([P, w], BF16, tag="opn")
                    nc.sync.dma_start(opn[:], scr[:, off2:off2 + w])
                    nrm = p2p.tile([P, w], BF16, tag="nrm")
                    if ti % 3 == 2:
                        aff = p2p.tile([P, w], BF16, tag="aff")
                        nc.vector.tensor_scalar(
                            aff[:], opn[:], scale[:], shift[:],
                            op0=ALU.mult, op1=ALU.add,
                        )
                        nc.vector.tensor_scalar_max(nrm[:], aff[:], 0.0)
                    else:
                        nc.scalar.activation(
                            nrm[:], opn[:], AF.Relu, bias=shift[:], scale=scale[:]
                        )
                    nc.sync.dma_start(out_t[:, off2:off2 + w], nrm[:])

    if not nc.is_finalized():
        nc.finalize()
    return nc


def _wrap16(a, S):
    """[n] int array -> dma_gather idx layout [128, S] int16 (zero-pad to 16*S)."""
    out = np.zeros((16, S), dtype=np.int16)
    n = a.shape[0]
    full = np.zeros(16 * S, dtype=np.int16)
    full[:n] = a.astype(np.int16)
    out = full.reshape(S, 16).T
    return np.tile(out, (8, 1))


def prepare(x, edge_index, edge_attr, w_self, w_h, w_t, gamma, beta_bn):
    """Host-side packing.  Returns (seg_chunks, in_maps, restore) where
    restore[core] = (positions array mapping padded row -> original edge)."""
    x = np.asarray(x, dtype=np.float32)
    xb = x.astype(BF)

    ei = np.asarray(edge_index)
    row_all = ei[0].astype(np.int64)
    col_all = ei[1].astype(np.int64)
    ea_all = np.asarray(edge_attr, dtype=np.float32)

    ws = np.ascontiguousarray(np.asarray(w_self, dtype=np.float32)).astype(BF)
    wh = (0.5 * np.asarray(w_h, dtype=np.float32)).astype(BF)
    wt = (0.5 * np.asarray(w_t, dtype=np.float32)).astype(BF)
    gm = np.ascontiguousarray(np.asarray(gamma, np.float32).reshape(C, 1))
    bt = np.ascontiguousarray(np.asarray(beta_bn, np.float32).reshape(C, 1))

    # per-core class split
    cores = []
    counts = np.zeros((N_CORES, 4), dtype=np.int64)
    for c in range(N_CORES):
        sl = slice(c * E_SHARD, (c + 1) * E_SHARD)
        row, col = row_all[sl], col_all[sl]
        cls = 2 * (row >= LO_ROWS).astype(np.int64) + (col >= LO_ROWS)
        order = np.argsort(cls, kind="stable")
        cores.append((row, col, cls, order, ea_all[sl]))
        counts[c] = np.bincount(cls, minlength=4)

    seg_sizes = tuple(
        int(-(-int(counts[:, k].max()) // SUB)) * SUB for k in range(4)
    )
    e_pad = sum(seg_sizes)
    chunk_widths = []
    for k in range(4):
        rem = seg_sizes[k]
        while rem > 0:
            chunk_widths.append(min(CHUNK, rem))
            rem -= min(CHUNK, rem)
    nchunk = len(chunk_widths)
    SMAX = CHUNK // 16
    seg_off = np.concatenate([[0], np.cumsum(np.array(seg_sizes))])

    in_maps = []
    restores = []
    for c in range(N_CORES):
        row, col, cls, order, ea = cores[c]
        ridx = np.zeros(e_pad, dtype=np.int64)
        cidx = np.zeros(e_pad, dtype=np.int64)
        eaP = np.zeros((e_pad, C), dtype=np.float32)
        rest = []  # (orig_ids, padded_start, n)
        for k in range(4):
            ids = order[cls[order] == k]
            n = len(ids)
            o0 = int(seg_off[k])
            r = row[ids] - (0 if k < 2 else HI_BASE)
            cc = col[ids] - (0 if k % 2 == 0 else HI_BASE)
            ridx[o0:o0 + n] = r
            cidx[o0:o0 + n] = cc
            eaP[o0:o0 + n] = ea[ids]
            rest.append((ids, o0, n))
        restores.append(rest)

        idxpack = np.zeros((nchunk, 2, P, SMAX), dtype=np.int16)
        a = 0
        for giq, chw in enumerate(chunk_widths):
            Sw = chw // 16
            idxpack[giq, 0, :, 0:Sw] = _wrap16(ridx[a:a + chw], Sw)
            idxpack[giq, 1, :, 0:Sw] = _wrap16(cidx[a:a + chw], Sw)
            a += chw
        assert a == e_pad

        eaT = np.ascontiguousarray(eaP.T.astype(BF))
        in_maps.append({
            "xb": xb,
            "eaT": eaT,
            "idxpack": idxpack,
            "w_self": ws,
            "w_h2": wh,
            "w_t2": wt,
            "gamma": gm,
            "beta": bt,
        })
    return seg_sizes, in_maps, restores


_NC_CACHE = {}


def _get_nc(seg_sizes):
    if seg_sizes not in _NC_CACHE:
        _NC_CACHE[seg_sizes] = build_nc(seg_sizes)
    return _NC_CACHE[seg_sizes]


def run(inputs, trace=False, **kwargs):
    from concourse.bass_utils import run_bass_kernel_spmd

    seg_sizes, in_maps, restores = prepare(
        inputs["x"], inputs["edge_index"], inputs["edge_attr"],
        inputs["w_self"], inputs["w_h"], inputs["w_t"],
        inputs["gamma"], inputs["beta_bn"],
    )
    nc = _get_nc(seg_sizes)
    res = run_bass_kernel_spmd(
        nc, in_maps, core_ids=list(range(N_CORES)), trace=trace, **kwargs
    )
    out = np.empty((N_EDGES, C), dtype=np.float32)
    for c in range(N_CORES):
        outT = np.asarray(res.results[c]["outT"])  # [C, e_pad] bf16
        outP = outT.T.astype(np.float32)           # [e_pad, C]
        base = c * E_SHARD
        for ids, o0, n in restores[c]:
            out[base + ids] = outP[o0:o0 + n]
    return out, res


def kernel(**inputs):
    out, _ = run(inputs, trace=False)
    return out
